# revision 2
# baseline (speedup 1.0000x reference)
"""Bass/Tile kernel for masked dot-product attention on 8 Trainium2 cores.

Problem: queries/keys/values [128, 1024, 64] fp32, valid_lens [128] int32.
  out[b] = softmax(mask(Q K^T / 8, valid_lens[b])) @ V

Strategy (v2 — host-precast fp16 panels):
  * Shard the 128 batch*heads across 8 cores, 16 head-slots per core.
    Heads are sorted by valid_len (descending) and dealt round-robin so
    every core gets the same per-slot chunk count -> one SPMD program.
  * Per head, only ceil(valid_len/128) key chunks contribute (the rest are
    fully masked -> softmax weight exactly 0); the program skips them.
  * Host pre-casts Q/K/V to fp16 and packs [Q;K] per head as a
    [2048, 128] panel (cols 0:64 = data, 64:128 = zeros).  One
    dma_start_transpose per head then yields the zero-padded Q^T/K^T
    SBUF panel directly — no on-chip casting or staging.
  * Compute S^T = K Q^T chunkwise on the PE ([128 k x 1024 q]), so the
    PV matmul consumes P^T directly as the moving operand.  Masking +
    1/sqrt(d) scaling + exp run as one ScalarE activation (bias =
    per-partition mask column of 0 / -1e6; exp(-1e6) underflows to 0).
  * Softmax denominators come free: a ones-column is appended to V's
    stationary panel, so PV accumulation produces [O^T ; sum_k P^T].
  * Epilogue: PE-transpose [O^T ; denom] back, normalize on DVE, store.
  * Heads with valid_len == 0 are fixed up on the host (mean of V).
"""

import math
from contextlib import ExitStack

import numpy as np

import concourse.bass as bass  # noqa: F401
import concourse.mybir as mybir
import concourse.tile as tile
from concourse import bacc
from concourse.bass_utils import run_bass_kernel_spmd
from concourse.masks import make_identity

BH, L, D = 128, 1024, 64
NCORES = 8
SLOTS = BH // NCORES  # 16
CHUNK = 128
NCH = L // CHUNK  # 8
PANEL_ROWS = 2 * L  # 2048: Q rows then K rows
MASK_VALUE = -1000000.0
F32 = mybir.dt.float32
MM_DT = mybir.dt.float16

_program_cache: dict = {}


def _build_program(m_list):
    nc = bacc.Bacc("TRN2", target_bir_lowering=False, debug=False)
    qk_d = nc.dram_tensor(
        "qk", [SLOTS, PANEL_ROWS, 128], MM_DT, kind="ExternalInput"
    ).ap()
    v_d = nc.dram_tensor("v", [SLOTS, L, D], MM_DT, kind="ExternalInput").ap()
    mb_d = nc.dram_tensor("mb", [CHUNK, SLOTS * NCH], F32, kind="ExternalInput").ap()
    o_d = nc.dram_tensor("o", [SLOTS, L, D], F32, kind="ExternalOutput").ap()

    Exp = mybir.ActivationFunctionType.Exp
    NQBUF = 4
    NVBUF = 4
    NPT = 4
    NS = 2

    with tile.TileContext(nc) as tc, ExitStack() as ctx:
        const = ctx.enter_context(tc.tile_pool(name="const", bufs=1))
        ident = const.tile([128, 128], F32)
        make_identity(nc, ident)
        mb = const.tile([CHUNK, SLOTS * NCH], F32)
        nc.sync.dma_start(mb[:], mb_d[:])
        ones = const.tile([128, 1], F32)
        nc.gpsimd.memset(ones[:], 1.0)
        actwarm = const.tile([128, 1], F32, tag="actwarm")
        nc.scalar.activation(actwarm[:], ones[:], Exp, bias=0.0, scale=1.0)

        qt_p = ctx.enter_context(tc.tile_pool(name="qt", bufs=1))
        vp_p = ctx.enter_context(tc.tile_pool(name="vp", bufs=1))
        pt_p = ctx.enter_context(tc.tile_pool(name="pt", bufs=1))
        ot_p = ctx.enter_context(tc.tile_pool(name="ot", bufs=1))
        osb_p = ctx.enter_context(tc.tile_pool(name="osb", bufs=1))
        rec_p = ctx.enter_context(tc.tile_pool(name="rec", bufs=1))

        s_ps = ctx.enter_context(tc.tile_pool(name="s", bufs=1, space="PSUM"))
        o_ps = ctx.enter_context(tc.tile_pool(name="ops", bufs=4, space="PSUM"))

        # Persistent buffers (manual rotation, so one-time inits survive).
        qkt_bufs = [
            qt_p.tile([128, PANEL_ROWS], MM_DT, tag=f"qkt{i}", name=f"qkt{i}")
            for i in range(NQBUF)
        ]
        vp_bufs = [
            vp_p.tile([128, NCH, D + 1], MM_DT, tag=f"vp{i}", name=f"vp{i}")
            for i in range(NVBUF)
        ]
        for i in range(NVBUF):
            nc.gpsimd.memset(vp_bufs[i][:, :, D : D + 1], 1.0)
        pt_bufs = [
            pt_p.tile([128, L], MM_DT, tag=f"pt{i}", name=f"pt{i}")
            for i in range(NPT)
        ]
        s_bufs = [
            s_ps.tile([128, L], F32, tag=f"s{i}", name=f"s{i}") for i in range(NS)
        ]

        # Dense matmul burst to flip the PE HAM clock-gate to full rate
        # (~3.4us of contiguous activity required) before real work starts.
        warm = const.tile([128, 512], MM_DT, tag="warm")
        nc.gpsimd.memset(warm[:], 0.5)
        wps = o_ps.tile([128, 512], F32, tag="ops")  # noqa
        for i in range(10):
            nc.tensor.matmul(wps[:], warm[:, 0:128], warm[:], start=True, stop=True)

        def load_head(j, m):
            nrows = L + m * CHUNK
            qkt = qkt_bufs[j % NQBUF]
            nc.sync.dma_start_transpose(qkt[:, 0:nrows], qk_d[j, 0:nrows, :])
            vp = vp_bufs[j % NVBUF]
            nc.gpsimd.dma_start(
                vp[:, 0:m, 0:D],
                v_d[j, 0 : m * CHUNK].rearrange("(c p) d -> p c d", p=CHUNK),
            )
            return qkt, vp

        chunk_ctr = 0
        epilogue_pending = []
        pending = [load_head(jj, m_list[jj]) for jj in range(min(3, SLOTS))]
        for j in range(SLOTS):
            m = m_list[j]
            qkt, vp = pending.pop(0)
            if j + 3 < SLOTS:
                pending.append(load_head(j + 3, m_list[j + 3]))

            opsum = [
                o_ps.tile([128, 512], F32, tag="ops", name=f"op{j}_{h}")
                for h in range(2)
            ]
            pts = {}

            def emit_pv(c):
                vl = vp[:, c, :]
                for h in range(2):
                    nc.tensor.matmul(
                        opsum[h][0:65, :],
                        vl,
                        pts[c][:, h * 512 : (h + 1) * 512],
                        start=(c == 0),
                        stop=(c == m - 1),
                    )

            for c in range(m):
                s = s_bufs[chunk_ctr % NS]
                chunk_ctr += 1
                for h in range(2):
                    nc.tensor.matmul(
                        s[:, h * 512 : (h + 1) * 512],
                        qkt[:, L + c * 128 : L + (c + 1) * 128],
                        qkt[:, h * 512 : (h + 1) * 512],
                        start=True,
                        stop=True,
                    )
                if c >= 1:
                    emit_pv(c - 1)
                if c == 1 and epilogue_pending:
                    epilogue_pending.pop(0)()
                pts[c] = pt_bufs[(chunk_ctr - 1) % NPT]
                col = j * NCH + c
                nc.scalar.activation(
                    pts[c][:], s[:], Exp, bias=mb[:, col : col + 1], scale=0.125
                )
            emit_pv(m - 1)
            if m == 1 and epilogue_pending:
                epilogue_pending.pop(0)()

            def make_epilogue(j, m, opsum):
                def epi():
                    # Transpose [O^T ; denom] back (4 blocks per PSUM bank),
                    # normalize, one store.
                    ot = ot_p.tile([65, L], F32, tag="ot", name=f"ot{j}")
                    for h in range(2):
                        nc.vector.tensor_copy(
                            ot[:, h * 512 : (h + 1) * 512], opsum[h][0:65, :]
                        )
                    osb = osb_p.tile([128, NCH * D], F32, tag="osb", name=f"osb{j}")
                    for gg in range(2):
                        tt = o_ps.tile(
                            [128, 4 * 65], F32, tag="ops", name=f"tt{j}_{gg}"
                        )
                        for g4 in range(4):
                            g = 4 * gg + g4
                            nc.tensor.transpose(
                                tt[:, g4 * 65 : g4 * 65 + 65],
                                ot[:, g * 128 : (g + 1) * 128],
                                ident[0:65, 0:65],
                            )
                        rec = rec_p.tile(
                            [128, 4], F32, tag="rec", name=f"rec{j}_{gg}"
                        )
                        nc.vector.reciprocal(
                            rec[:],
                            tt[:].rearrange("p (g e) -> p g e", e=65)[:, :, 64],
                        )
                        for g4 in range(4):
                            g = 4 * gg + g4
                            nc.vector.tensor_scalar_mul(
                                osb[:, g * D : (g + 1) * D],
                                tt[:, g4 * 65 : g4 * 65 + 64],
                                rec[:, g4 : g4 + 1],
                            )
                    nc.gpsimd.dma_start(
                        o_d[j].rearrange("(g p) d -> p g d", p=CHUNK),
                        osb[:].rearrange("p (g d) -> p g d", d=D),
                    )

                return epi

            epilogue_pending.append(make_epilogue(j, m, opsum))

        for epi in epilogue_pending:
            epi()

    nc.compile()
    return nc


def _plan(valid_lens):
    """Sort heads by valid_len desc, deal round-robin across cores.

    Returns (assign [NCORES, SLOTS] head indices, m_list [SLOTS] chunk counts).
    """
    order = np.argsort(-valid_lens, kind="stable")
    assign = order.reshape(SLOTS, NCORES).T  # [core, slot]
    m_list = []
    for j in range(SLOTS):
        vmax = int(valid_lens[assign[:, j]].max())
        m_list.append(min(NCH, max(1, math.ceil(vmax / CHUNK))))
    return assign, m_list


def _run(queries, keys, values, valid_lens, trace=False):
    queries = np.asarray(queries, dtype=np.float32)
    keys = np.asarray(keys, dtype=np.float32)
    values = np.asarray(values, dtype=np.float32)
    valid_lens = np.asarray(valid_lens, dtype=np.int32)

    assign, m_list = _plan(valid_lens)

    key = tuple(m_list)
    nc = _program_cache.get(key)
    if nc is None:
        nc = _build_program(m_list)
        _program_cache[key] = nc

    q16 = queries.astype(np.float16)
    k16 = keys.astype(np.float16)
    v16 = values.astype(np.float16)

    kk = np.arange(L, dtype=np.int64)
    in_maps = []
    for i in range(NCORES):
        heads = assign[i]
        panel = np.zeros((SLOTS, PANEL_ROWS, 128), dtype=np.float16)
        panel[:, 0:L, 0:D] = q16[heads]
        for jj, (h, m) in enumerate(zip(heads, m_list)):
            panel[jj, L : L + m * CHUNK, 0:D] = k16[h, 0 : m * CHUNK]
        mask = np.where(
            kk[None, :] < valid_lens[heads][:, None], 0.0, MASK_VALUE
        ).astype(np.float32)  # [SLOTS, L]
        # mb[p, j*NCH+c] = mask for key index c*128+p of slot j.
        mb = np.transpose(mask.reshape(SLOTS, NCH, CHUNK), (2, 0, 1)).reshape(
            CHUNK, SLOTS * NCH
        )
        in_maps.append(
            {
                "qk": panel,
                "v": v16[heads],
                "mb": np.ascontiguousarray(mb),
            }
        )

    res = run_bass_kernel_spmd(nc, in_maps, list(range(NCORES)), trace=trace)

    out = np.empty((BH, L, D), dtype=np.float32)
    for i in range(NCORES):
        out[assign[i]] = res.results[i]["o"]

    # valid_len == 0: reference softmaxes an all-masked row -> uniform weights.
    for h in np.nonzero(valid_lens == 0)[0]:
        out[h] = values[h].mean(axis=0, keepdims=True)

    return out, res


def kernel(queries, keys, values, valid_lens):
    out, _ = _run(queries, keys, values, valid_lens)
    return out
